# revision 33
# baseline (speedup 1.0000x reference)
"""Trainium2 Bass kernel for nn_AutoregressiveDecoder (8-core data parallel).

Strategy:
  - Pure data parallel: B=16384 rows sharded 2048/core across 8 NeuronCores.
  - MLP compute runs feature-major (features on partitions, batch on the free
    dim) in bf16 so weights act as the matmul stationary operand.
  - seq_embed @ w1[:512] is step-invariant -> computed once per 512-row
    macro-tile ("base"), per-step only the small state/onehot extra columns
    are matmul'd and added.
  - seq ships host-pre-transposed [D, BL] so seqT loads are contiguous.
  - Per-row scalar plumbing (losses, bce, clips, selects, state scatter) runs
    in a "blocked" batch-major layout [32 partitions, 16 blocks x 32 slots],
    bridged to/from feature-major with single-instruction 32x32
    StreamTransposes on the vector engine.
  - Index-only preprocessing (ALL_PERMS lookup, one-hot, take_along_axis
    gathers) happens host-side in numpy; loss partial sums are reduced
    host-side (psum of 4 scalars x 8 cores).
"""

import numpy as np
import ml_dtypes

import concourse.bass as bass
import concourse.bacc as bacc
import concourse.tile as tile
from concourse import mybir
from concourse.bass_utils import run_bass_kernel_spmd

BF16 = mybir.dt.bfloat16
F32 = mybir.dt.float32
AF = mybir.ActivationFunctionType
ALU = mybir.AluOpType
NP_BF16 = ml_dtypes.bfloat16

B, D, H = 16384, 512, 512
NCORES = 8
NB = 512            # macro-tile rows (matmul free dim)
ALL_PERMS = np.array(
    [[0, 1, 2], [0, 2, 1], [1, 0, 2], [1, 2, 0], [2, 0, 1], [2, 1, 0]], np.int32
)

# blocked-layout slot map (32 slots per 32-row block)
S_P, S_FL, S_ROH, S_F, S_E = 0, 3, 6, 9, 12   # each 3 wide


def r3(t, s):
    """view a [32, 16*s] tile as [32 p, 16 j, s slots]"""
    return t[:, :].rearrange("p (j s) -> p j s", s=s)


def _enable_ldw_opt():
    """walrus --enable-ldw-opt=false is hardcoded; flip it (dedups LDWEIGHTS)."""
    from concourse import bass_utils as bu
    if getattr(bu, "_ldw_patched", False):
        return
    orig = bu.run_command

    def patched(cmd, *a, **k):
        cmd = list(cmd)  # ldw-opt=true crashes walrus on this BIR; keep off
        return orig(cmd, *a, **k)

    bu.run_command = patched
    bu._ldw_patched = True


def build_graph(BL):
    """Build the per-core Bass graph. BL = rows per core (multiple of NB)."""
    _enable_ldw_opt()
    NM = BL // NB          # macro-tiles per core
    NBLK = NB // 32        # 32-row blocks per macro-tile (16)
    BLKT = BL // 32        # total blocks per core

    nc = bacc.Bacc("TRN2", target_bir_lowering=False, debug=False,
                   num_devices=NCORES)

    # ---- dram parameters -------------------------------------------------
    U8 = mybir.dt.uint8
    seq_d = nc.dram_tensor("seq", [D, BL], BF16, kind="ExternalInput").ap()
    gts_d = nc.dram_tensor("gts", [96, BLKT * 3], F32, kind="ExternalInput").ap()
    roh_d = nc.dram_tensor("roh", [96, BLKT * 3], F32, kind="ExternalInput").ap()
    # uint8 copies of the masks (CopyPredicated wants integer predicates)
    mi_d = nc.dram_tensor("mi", [96, BLKT], U8, kind="ExternalInput").ap()
    rohi_d = nc.dram_tensor("rohi", [96, BLKT * 3], U8, kind="ExternalInput").ap()

    pw1_d = nc.dram_tensor("pw1", [D, H], BF16, kind="ExternalInput").ap()
    pw1x_d = nc.dram_tensor("pw1x", [9, H], BF16, kind="ExternalInput").ap()
    pb1_d = nc.dram_tensor("pb1", [H], F32, kind="ExternalInput").ap()
    pw2_d = nc.dram_tensor("pw2", [H, H // 2], BF16, kind="ExternalInput").ap()
    pb2_d = nc.dram_tensor("pb2", [H // 2], F32, kind="ExternalInput").ap()
    pw3_d = nc.dram_tensor("pw3p", [H // 2, 32], BF16, kind="ExternalInput").ap()

    fw1_d = nc.dram_tensor("fw1", [D, H], BF16, kind="ExternalInput").ap()
    fw1x_d = nc.dram_tensor("fw1x", [15, H], BF16, kind="ExternalInput").ap()
    fb1_d = nc.dram_tensor("fb1", [H], F32, kind="ExternalInput").ap()
    fw2_d = nc.dram_tensor("fw2", [H, H], BF16, kind="ExternalInput").ap()
    fb2_d = nc.dram_tensor("fb2", [H], F32, kind="ExternalInput").ap()
    fw3_d = nc.dram_tensor("fw3p", [H, 32], BF16, kind="ExternalInput").ap()
    eye_d = nc.dram_tensor("eye", [128, 128], BF16, kind="ExternalInput").ap()
    b3s_d = nc.dram_tensor("b3s", [1, 3], F32, kind="ExternalInput").ap()

    df_d = nc.dram_tensor("df", [BL, 3], F32, kind="ExternalOutput").ap()
    dp_d = nc.dram_tensor("dp", [BL, 3], F32, kind="ExternalOutput").ap()
    de_d = nc.dram_tensor("de", [BL, 3], F32, kind="ExternalOutput").ap()
    # raw per-step MLP outputs (loss terms are finished host-side)
    lg_d = nc.dram_tensor("lgo", [BL, 3], F32, kind="ExternalOutput").ap()
    pf_d = nc.dram_tensor("pfo", [BL, 3], F32, kind="ExternalOutput").ap()
    pe_d = nc.dram_tensor("peo", [BL, 3], F32, kind="ExternalOutput").ap()

    v = nc.vector
    sc = nc.scalar
    te = nc.tensor

    with tile.TileContext(nc) as tc:
        wpool = tc.alloc_tile_pool(name="w", bufs=1)
        pers = tc.alloc_tile_pool(name="pers", bufs=1)
        bigp = tc.alloc_tile_pool(name="big", bufs=3)
        stp = tc.alloc_tile_pool(name="stp", bufs=2)
        smp = tc.alloc_tile_pool(name="smp", bufs=4)
        psp = tc.alloc_tile_pool(name="psum", bufs=6, space="PSUM")
        ps3 = tc.alloc_tile_pool(name="psum3", bufs=2, space="PSUM")

        # ---- load weights (persistent) ----------------------------------
        pw1_sb = wpool.tile([128, 4 * H], BF16)
        fw1_sb = wpool.tile([128, 4 * H], BF16)
        pw2_sb = wpool.tile([128, 4 * (H // 2)], BF16)
        fw2_sb = wpool.tile([128, 4 * H], BF16)
        for k in range(4):
            nc.gpsimd.dma_start(pw1_sb[:, k * H:(k + 1) * H],
                                pw1_d[k * 128:(k + 1) * 128, :])
            nc.gpsimd.dma_start(fw1_sb[:, k * H:(k + 1) * H],
                                fw1_d[k * 128:(k + 1) * 128, :])
            nc.gpsimd.dma_start(pw2_sb[:, k * 256:(k + 1) * 256],
                                pw2_d[k * 128:(k + 1) * 128, :])
            nc.gpsimd.dma_start(fw2_sb[:, k * H:(k + 1) * H],
                                fw2_d[k * 128:(k + 1) * 128, :])
        pw1x_sb = wpool.tile([9, H], BF16)
        fw1x_sb = wpool.tile([15, H], BF16)
        nc.gpsimd.dma_start(pw1x_sb[:, :], pw1x_d[:, :])
        nc.gpsimd.dma_start(fw1x_sb[:, :], fw1x_d[:, :])
        pw3_sb = wpool.tile([128, 64], BF16)   # 2 k-chunks x 32 cols
        fw3_sb = wpool.tile([128, 128], BF16)  # 4 k-chunks x 32 cols
        for k in range(2):
            nc.gpsimd.dma_start(pw3_sb[:, k * 32:(k + 1) * 32],
                                pw3_d[k * 128:(k + 1) * 128, :])
        for k in range(4):
            nc.gpsimd.dma_start(fw3_sb[:, k * 32:(k + 1) * 32],
                                fw3_d[k * 128:(k + 1) * 128, :])

        # biases as [128, nchunk] f32 (partition p, chunk m) for ACT bias APs
        pb1_sb = wpool.tile([128, 4], F32)
        fb1_sb = wpool.tile([128, 4], F32)
        fb2_sb = wpool.tile([128, 4], F32)
        pb2_sb = wpool.tile([128, 2], F32)
        nc.gpsimd.dma_start(pb1_sb[:, :],
                            pb1_d.rearrange("(m p) -> p m", p=128))
        nc.gpsimd.dma_start(fb1_sb[:, :],
                            fb1_d.rearrange("(m p) -> p m", p=128))
        nc.gpsimd.dma_start(fb2_sb[:, :],
                            fb2_d.rearrange("(m p) -> p m", p=128))
        nc.gpsimd.dma_start(pb2_sb[:, :],
                            pb2_d.rearrange("(m p) -> p m", p=128))
        eye_sb = wpool.tile([128, 128], BF16)
        nc.gpsimd.dma_start(eye_sb[:, :], eye_d[:, :])
        # l3 output biases broadcast to 32 partitions: cols (pb3, fb3_0, fb3_1)
        b3s_sb = wpool.tile([1, 3], F32)
        nc.gpsimd.dma_start(b3s_sb[:, :], b3s_d[:, :])
        b3bc = wpool.tile([32, 3], F32)
        nc.gpsimd.partition_broadcast(b3bc[:, :], b3s_sb[:, :])

        # ---- phase 1: all macro-tile bases (PE-dense, keeps HAM warm) ---
        base_p, base_f, sts, lgos, pfos, peos = {}, {}, {}, {}, {}, {}
        for mt in range(NM):
            rows = slice(mt * NB, (mt + 1) * NB)
            seqT = []
            for k in range(4):
                t = stp.tile([128, NB], BF16, tag=f"seqT{k}")
                nc.sync.dma_start(t[:, :], seq_d[k * 128:(k + 1) * 128, rows])
                seqT.append(t)
            base_p[mt] = pers.tile([128, 4 * NB], BF16, tag=f"base_p{mt}", name=f"base_p{mt}")
            base_f[mt] = pers.tile([128, 4 * NB], BF16, tag=f"base_f{mt}", name=f"base_f{mt}")
            for m in range(4):
                pp = psp.tile([128, NB], F32, tag="ps_main")
                for k in range(4):
                    te.matmul(pp[:, :],
                              pw1_sb[:, k * H + m * 128: k * H + (m + 1) * 128],
                              seqT[k][:, :], start=(k == 0), stop=(k == 3))
                sc.activation(base_p[mt][:, m * NB:(m + 1) * NB], pp[:, :],
                              AF.Identity, bias=pb1_sb[:, m:m + 1])
                pf_ = psp.tile([128, NB], F32, tag="ps_main")
                for k in range(4):
                    te.matmul(pf_[:, :],
                              fw1_sb[:, k * H + m * 128: k * H + (m + 1) * 128],
                              seqT[k][:, :], start=(k == 0), stop=(k == 3))
                sc.activation(base_f[mt][:, m * NB:(m + 1) * NB], pf_[:, :],
                              AF.Identity, bias=fb1_sb[:, m:m + 1])
            sts[mt] = pers.tile([32, NBLK * 32], F32, tag=f"st{mt}", name=f"st{mt}")
            v.memset(sts[mt][:, :], 0.0)
            lgos[mt] = pers.tile([32, NBLK * 3], F32, tag=f"lgo{mt}", name=f"lgo{mt}")
            pfos[mt] = pers.tile([32, NBLK * 3], F32, tag=f"pfo{mt}", name=f"pfo{mt}")
            peos[mt] = pers.tile([32, NBLK * 3], F32, tag=f"peo{mt}", name=f"peo{mt}")

        # ---- phase 2: autoregressive steps, macro-interleaved -----------
        # While macro mt's plumbing runs on DVE/ACT, the PE executes the
        # other macros' matmuls (engines are in-order; interleaved emission
        # is what lets the scheduler fill the gaps).
        for s in range(3):
            for mt in range(NM):
                st3 = r3(sts[mt], 32)
                # fresh one-hot for this step into slots 6..8
                nc.gpsimd.dma_start(
                    st3[:, :, S_ROH:S_ROH + 3],
                    roh_d[s * 32:(s + 1) * 32,
                          mt * NBLK * 3:(mt + 1) * NBLK * 3].rearrange(
                              "p (j r) -> p j r", r=3))
                gt_sb = smp.tile([32, NBLK * 3], F32, tag="gt")
                nc.gpsimd.dma_start(
                    gt_sb[:, :],
                    gts_d[s * 32:(s + 1) * 32,
                          mt * NBLK * 3:(mt + 1) * NBLK * 3])
                gt3 = r3(gt_sb, 3)
                gtf, gtp, gte = gt3[:, :, 0:1], gt3[:, :, 1:2], gt3[:, :, 2:3]
                mi_sb = smp.tile([32, NBLK], U8, tag="mi")
                nc.gpsimd.dma_start(
                    mi_sb[:, :],
                    mi_d[s * 32:(s + 1) * 32, mt * NBLK:(mt + 1) * NBLK])
                msk = r3(mi_sb, 1)[:, :, 0:1]
                roi_sb = smp.tile([32, NBLK * 3], U8, tag="roi")
                nc.gpsimd.dma_start(
                    roi_sb[:, :],
                    rohi_d[s * 32:(s + 1) * 32,
                           mt * NBLK * 3:(mt + 1) * NBLK * 3])
                roi3 = r3(roi_sb, 3)

                # bridge to feature-major: cast + stream-transpose
                st_bf = smp.tile([32, NBLK * 32], BF16, tag="stbf")
                v.tensor_copy(st_bf[:, :], sts[mt][:, :])
                exT = smp.tile([32, NBLK * 32], BF16, tag="exT")
                v.transpose(exT[:, :], st_bf[:, :])

                # layer 1: base + extra @ w1x -> gelu -> h1 (bf16)
                h1p = bigp.tile([128, 4 * NB], BF16, tag="h1p")
                h1f = bigp.tile([128, 4 * NB], BF16, tag="h1f")
                for m in range(4):
                    pp = psp.tile([128, NB], F32, tag="ps_main")
                    te.matmul(pp[:, :], pw1x_sb[:, m * 128:(m + 1) * 128],
                              exT[0:9, :], start=True, stop=True)
                    v.tensor_add(pp[:, :], pp[:, :],
                                 base_p[mt][:, m * NB:(m + 1) * NB])
                    sc.activation(h1p[:, m * NB:(m + 1) * NB], pp[:, :],
                                  AF.Gelu)
                    pf_ = psp.tile([128, NB], F32, tag="ps_main")
                    te.matmul(pf_[:, :], fw1x_sb[:, m * 128:(m + 1) * 128],
                              exT[0:15, :], start=True, stop=True)
                    v.tensor_add(pf_[:, :], pf_[:, :],
                                 base_f[mt][:, m * NB:(m + 1) * NB])
                    sc.activation(h1f[:, m * NB:(m + 1) * NB], pf_[:, :],
                                  AF.Gelu)

                # layer 2
                h2p = bigp.tile([128, 2 * NB], BF16, tag="h2p")
                for m in range(2):
                    pp = psp.tile([128, NB], F32, tag="ps_main")
                    for k in range(4):
                        te.matmul(pp[:, :],
                                  pw2_sb[:, k * 256 + m * 128:
                                         k * 256 + (m + 1) * 128],
                                  h1p[:, k * NB:(k + 1) * NB],
                                  start=(k == 0), stop=(k == 3))
                    sc.activation(h2p[:, m * NB:(m + 1) * NB], pp[:, :],
                                  AF.Gelu, bias=pb2_sb[:, m:m + 1])
                h2f = bigp.tile([128, 4 * NB], BF16, tag="h2f")
                for m in range(4):
                    pp = psp.tile([128, NB], F32, tag="ps_main")
                    for k in range(4):
                        te.matmul(pp[:, :],
                                  fw2_sb[:, k * H + m * 128:
                                         k * H + (m + 1) * 128],
                                  h1f[:, k * NB:(k + 1) * NB],
                                  start=(k == 0), stop=(k == 3))
                    sc.activation(h2f[:, m * NB:(m + 1) * NB], pp[:, :],
                                  AF.Gelu, bias=fb2_sb[:, m:m + 1])

                # layer 3 (padded to M=32; pres in col-group 0, fe in
                # col-group 1 of the same psum tile -> MMs run concurrently)
                p3 = ps3.tile([64, NB], F32, tag="ps3")
                for k in range(2):
                    te.matmul(p3[0:32, :], pw3_sb[:, k * 32:(k + 1) * 32],
                              h2p[:, k * NB:(k + 1) * NB],
                              start=(k == 0), stop=(k == 1))
                for k in range(4):
                    te.matmul(p3[32:64, :], fw3_sb[:, k * 32:(k + 1) * 32],
                              h2f[:, k * NB:(k + 1) * NB],
                              start=(k == 0), stop=(k == 3),
                              tile_position=(0, 32))

                # bridge back to blocked batch-major; add b3 biases after
                lgT = smp.tile([32, NBLK * 32], F32, tag="lgT")
                v.transpose(lgT[:, :], p3[0:32, :])
                feT = smp.tile([32, NBLK * 32], F32, tag="feT")
                v.transpose(feT[:, :], p3[32:64, :])
                lg3 = r3(lgT, 32)
                fe3 = r3(feT, 32)
                logit = lg3[:, :, 0:1]
                pf = fe3[:, :, 0:1]
                pe = fe3[:, :, 1:2]
                v.tensor_scalar_add(logit, logit, b3bc[:, 0:1])
                v.tensor_scalar_add(pf, pf, b3bc[:, 1:2])
                v.tensor_scalar_add(pe, pe, b3bc[:, 2:3])

                # ---- plumbing (all [32, NBLK, 1] APs) -------------------
                pb = smp.tile([32, NBLK * 8], F32, tag="pb")
                pb3d = r3(pb, 8)
                a_f, a_p, a_e = pb3d[:, :, 0:1], pb3d[:, :, 1:2], pb3d[:, :, 2:3]
                sig, pfc, pec = pb3d[:, :, 3:4], pb3d[:, :, 4:5], pb3d[:, :, 5:6]

                # raw outputs for host-side loss
                v.tensor_copy(r3(lgos[mt], 3)[:, :, s:s + 1], logit)
                v.tensor_copy(r3(pfos[mt], 3)[:, :, s:s + 1], pf)
                v.tensor_copy(r3(peos[mt], 3)[:, :, s:s + 1], pe)

                # sigmoid(l) = 0.5*tanh(0.5*l) + 0.5 (stays on the gelu table)
                sc.activation(sig, logit, AF.Tanh, scale=0.5)
                v.tensor_scalar(sig, sig, 0.5, 0.5, ALU.mult, ALU.add)
                v.tensor_scalar(pfc, pf, -10.0, 10.0, ALU.max, ALU.min)
                v.tensor_scalar(pec, pe, -100.0, 100.0, ALU.max, ALU.min)
                v.tensor_copy(a_f, gtf)
                v.copy_predicated(a_f, msk, pfc)
                v.tensor_copy(a_p, gtp)
                v.copy_predicated(a_p, msk, sig)
                v.tensor_copy(a_e, gte)
                v.copy_predicated(a_e, msk, pec)

                # state scatter: slot = act where roh_r else keep
                for r in range(3):
                    rp = roi3[:, :, r:r + 1]
                    v.copy_predicated(st3[:, :, S_P + r:S_P + r + 1], rp, a_p)
                    v.tensor_max(st3[:, :, S_FL + r:S_FL + r + 1],
                                 st3[:, :, S_FL + r:S_FL + r + 1],
                                 st3[:, :, S_ROH + r:S_ROH + r + 1])
                    v.copy_predicated(st3[:, :, S_F + r:S_F + r + 1], rp, a_f)
                    v.copy_predicated(st3[:, :, S_E + r:S_E + r + 1], rp, a_e)

                if s == 2:
                    rows = slice(mt * NB, (mt + 1) * NB)
                    nc.gpsimd.dma_start(
                        df_d[rows, :].rearrange("(j p) r -> p j r", p=32),
                        st3[:, :, S_F:S_F + 3])
                    nc.gpsimd.dma_start(
                        dp_d[rows, :].rearrange("(j p) r -> p j r", p=32),
                        st3[:, :, S_P:S_P + 3])
                    nc.gpsimd.dma_start(
                        de_d[rows, :].rearrange("(j p) r -> p j r", p=32),
                        st3[:, :, S_E:S_E + 3])
                    nc.gpsimd.dma_start(
                        lg_d[rows, :].rearrange("(j p) r -> p j r", p=32),
                        r3(lgos[mt], 3)[:, :, :])
                    nc.gpsimd.dma_start(
                        pf_d[rows, :].rearrange("(j p) r -> p j r", p=32),
                        r3(pfos[mt], 3)[:, :, :])
                    nc.gpsimd.dma_start(
                        pe_d[rows, :].rearrange("(j p) r -> p j r", p=32),
                        r3(peos[mt], 3)[:, :, :])

        # ---- outputs (unreachable marker) -------------------------------
        for mt in []:
            rows = slice(mt * NB, (mt + 1) * NB)
            st3 = r3(sts[mt], 32)
            nc.gpsimd.dma_start(
                df_d[rows, :].rearrange("(j p) r -> p j r", p=32),
                st3[:, :, S_F:S_F + 3])
            nc.gpsimd.dma_start(
                dp_d[rows, :].rearrange("(j p) r -> p j r", p=32),
                st3[:, :, S_P:S_P + 3])
            nc.gpsimd.dma_start(
                de_d[rows, :].rearrange("(j p) r -> p j r", p=32),
                st3[:, :, S_E:S_E + 3])
            nc.gpsimd.dma_start(
                lg_d[rows, :].rearrange("(j p) r -> p j r", p=32),
                r3(lgos[mt], 3)[:, :, :])
            nc.gpsimd.dma_start(
                pf_d[rows, :].rearrange("(j p) r -> p j r", p=32),
                r3(pfos[mt], 3)[:, :, :])
            nc.gpsimd.dma_start(
                pe_d[rows, :].rearrange("(j p) r -> p j r", p=32),
                r3(peos[mt], 3)[:, :, :])

        for p in (ps3, psp, smp, stp, bigp, pers, wpool):
            p.release()

    nc.compile()
    return nc


# ---------------------------------------------------------------------------
def prep_inputs(seq_embed, freq, pres, enrich,
                pw1, pb1, pw2, pb2, pw3, pb3,
                fw1, fb1, fw2, fb2, fw3, fb3,
                perm_idx, round_mask, BL):
    """Host-side (numpy) sharding + index preprocessing."""
    f32 = np.float32
    seq = np.asarray(seq_embed, f32)
    perms = ALL_PERMS[np.asarray(perm_idx)]                    # [B,3]
    gtf = np.take_along_axis(np.asarray(freq, f32), perms, 1)   # [B,3] (col=s)
    gtp = np.take_along_axis(np.asarray(pres, f32), perms, 1)
    gte = np.take_along_axis(np.asarray(enrich, f32), perms, 1)
    m = np.take_along_axis(np.asarray(round_mask), perms, 1).astype(f32)
    roh = (perms[:, :, None] == np.arange(3)[None, None, :]).astype(f32)  # [B,3s,3r]

    bf = lambda a: np.ascontiguousarray(np.asarray(a, f32).astype(NP_BF16))
    pw1x = bf(np.asarray(pw1, f32)[512:521][[0, 2, 4, 1, 3, 5, 6, 7, 8]])
    fw1x = bf(np.asarray(fw1, f32)[512:527][
        [1, 5, 9, 3, 7, 11, 12, 13, 14, 0, 4, 8, 2, 6, 10]])
    pw3p = np.zeros((256, 32), f32); pw3p[:, 0] = np.asarray(pw3, f32)[:, 0]
    fw3p = np.zeros((512, 32), f32); fw3p[:, 0:2] = np.asarray(fw3, f32)
    b3s = np.array([[np.asarray(pb3, f32)[0],
                     np.asarray(fb3, f32)[0], np.asarray(fb3, f32)[1]]], f32)

    shared = {
        "pw1": bf(np.asarray(pw1, f32)[:512]), "pw1x": pw1x,
        "pb1": np.ascontiguousarray(np.asarray(pb1, f32)),
        "pw2": bf(pw2), "pb2": np.ascontiguousarray(np.asarray(pb2, f32)),
        "pw3p": bf(pw3p),
        "fw1": bf(np.asarray(fw1, f32)[:512]), "fw1x": fw1x,
        "fb1": np.ascontiguousarray(np.asarray(fb1, f32)),
        "fw2": bf(fw2), "fb2": np.ascontiguousarray(np.asarray(fb2, f32)),
        "fw3p": bf(fw3p),
        "eye": np.eye(128, dtype=NP_BF16),
        "b3s": b3s,
    }

    in_maps = []
    ncores = seq.shape[0] // BL
    BLKT = BL // 32
    for c in range(ncores):
        rs = slice(c * BL, (c + 1) * BL)
        # blocked layouts: index [s*32+p, Jg*w + q], b_local = 32*Jg + p
        gt3 = np.stack([gtf[rs], gtp[rs], gte[rs]], -1)          # [BL,3s,3]
        gt3 = gt3.reshape(BLKT, 32, 3, 3).transpose(2, 1, 0, 3)  # [3s,32,J,3]
        rohc = roh[rs].reshape(BLKT, 32, 3, 3).transpose(2, 1, 0, 3)
        mc = m[rs].reshape(BLKT, 32, 3).transpose(2, 1, 0)       # [3s,32,J]
        in_maps.append(dict(
            seq=np.ascontiguousarray(seq[rs].astype(NP_BF16).T),
            gts=np.ascontiguousarray(gt3.reshape(3 * 32, BLKT * 3)),
            roh=np.ascontiguousarray(rohc.reshape(3 * 32, BLKT * 3)),
            mi=np.ascontiguousarray(mc.reshape(3 * 32, BLKT).astype(np.uint8)),
            rohi=np.ascontiguousarray(
                rohc.reshape(3 * 32, BLKT * 3).astype(np.uint8)),
            **shared))
    aux = dict(gtf=gtf, gtp=gtp, gte=gte, m=m)
    return in_maps, aux


def assemble(results, aux):
    """Gather per-core outputs; finish the (tiny) loss reductions host-side."""
    f32 = np.float32
    df = np.concatenate([r["df"] for r in results], 0).astype(f32)
    dp = np.concatenate([r["dp"] for r in results], 0).astype(f32)
    de = np.concatenate([r["de"] for r in results], 0).astype(f32)
    lg = np.concatenate([r["lgo"] for r in results], 0).astype(f32)
    pf = np.concatenate([r["pfo"] for r in results], 0).astype(f32)
    pe = np.concatenate([r["peo"] for r in results], 0).astype(f32)
    m, gtf, gtp, gte = aux["m"], aux["gtf"], aux["gtp"], aux["gte"]
    lf = np.sum(np.square(pf - gtf) * m, dtype=np.float64)
    le = np.sum(np.square(pe - gte) * m, dtype=np.float64)
    bce = (np.maximum(lg, 0.0) - lg * gtp
           + np.log1p(np.exp(-np.abs(lg), dtype=np.float64)))
    lp = np.sum(bce * m, dtype=np.float64)
    nm = np.sum(m, dtype=np.float64) + 1e-8
    head = np.array([lf / nm, lp / nm, le / nm], f32)
    return np.concatenate([head, df.ravel(), dp.ravel(), de.ravel()])


_CACHE = {}


def _get_graph(BL):
    if BL not in _CACHE:
        _CACHE[BL] = build_graph(BL)
    return _CACHE[BL]


def _install_profile_hook():
    """Provide antenv.axon_hooks (missing in this image) so trace=True works."""
    import sys, types
    try:
        import antenv.axon_hooks  # noqa: F401
        return
    except ImportError:
        pass
    from trn_agent_boot.trn_boot import _ntff_profile_via_ctypes
    hook = _ntff_profile_via_ctypes('/opt/axon/libaxon_pjrt.so')
    mod = types.ModuleType('antenv.axon_hooks')
    mod._hook = hook
    mod.get_axon_ntff_profile_hook = lambda: mod._hook
    mod.set_axon_ntff_profile_hook = lambda h: setattr(mod, '_hook', h)
    sys.modules['antenv.axon_hooks'] = mod


def run(inputs, trace=False):
    if trace:
        _install_profile_hook()
    BL = inputs["seq_embed"].shape[0] // NCORES
    nc = _get_graph(BL)
    in_maps, aux = prep_inputs(**inputs, BL=BL)
    res = run_bass_kernel_spmd(nc, in_maps, core_ids=list(range(NCORES)),
                               trace=trace)
    out = assemble(res.results, aux)
    return out, res


def kernel(**inputs):
    inputs = {k: np.asarray(v) for k, v in inputs.items()}
    out, _ = run(inputs)
    return out
